# revision 8
# baseline (speedup 1.0000x reference)
"""Distributed MixLoss (ArcFace-style margin CE) kernel for 8 Trainium2 cores.

Strategy (classification tensor-parallel, per sharding hint):
  - Shard weight [100000, 512] along num_classes: 12500 classes/core
    (padded to 12544 = 98*128 with zero rows).
  - Each core: L2-normalize its weight shard rows + the (replicated)
    input rows on device, compute G = (32 * x_n) @ w_n.T via PE matmul,
    stream the [512, 12544] logits shard back to DRAM, and accumulate
    per-row sum(exp(logit)) on the ScalarE (fused accum_out).
  - Host: gathers shards, patches the 512 label entries (margin M2=0.5
    via the angle-addition identity, no arccos needed on device),
    adjusts the per-row exp-sums for the patch, computes the scalar
    loss and the [512,1] target stats.  All host math is O(batch).

Matmul precision modes (PE fp32 is 4 cyc/row; float32r is 1 cyc/row but
~tf32 precision, measured 1.6e-4 rel):
  - "split" (default): x and w are split into f32r hi + lo halves after
    the on-chip transpose; G = xh@wh + xh@wl + xl@wh at 3 cyc/row total
    gives fp32-class accuracy (~1e-6) at 1/4 the fp32 PE cost... 3/4.
  - "f32r": single pass, fastest, ~1.6e-4 rel error.
  - "f32": plain fp32, 4 cyc/row, exact.

Numerical notes:
  - No row max is needed for the softmax: logits are in [-32, 32] so
    exp() stays in fp32 normal range; log_softmax = x - log(sum(exp x)).
  - Reference clips cos to +-(1 - 1e-6); with this input distribution
    |cos| < 0.7 so the clip never fires on the bulk matrix; the 512
    label entries get the exact clipped treatment on the host.
  - rsqrt for the norms: seeded as exp(-0.5*ln(ss)) on ACT (Ln and Exp
    share one activation-table set, so no table-swap stalls against the
    softmax Exp) + 2 Newton steps on DVE.  ACT sqrt (5.7e-3 rel on HW)
    and tensor_tensor_reduce (crashes on HW via this path) are avoided.
"""

import math
from contextlib import ExitStack

import numpy as np

import concourse.bass as bass
import concourse.bacc as bacc
import concourse.mybir as mybir
import concourse.tile as tile
from concourse.bass_utils import run_bass_kernel_spmd
from concourse.masks import make_identity

F32 = mybir.dt.float32
F32R = mybir.dt.float32r
AX = mybir.AxisListType
ALU = mybir.AluOpType
ACT_FN = mybir.ActivationFunctionType

B = 512          # batch
F = 512          # features
C = 100000       # classes
NCORES = 8
CS = C // NCORES          # 12500 valid classes per core
CSP = 12544               # padded classes per core (98 * 128)
W_CHUNK = 512             # class chunk width for matmul moving dim
NCHUNK = (CSP + W_CHUNK - 1) // W_CHUNK   # 25 (24 full + 1 of 256)
S = 32.0
M2 = 0.5
MM = M2 * math.sin(math.pi - M2)

MODE = "split"            # "split" | "f32r" | "f32"

_cached = {}


def _build_program(mode=MODE, nchunks=NCHUNK, no_gpsimd=False):
    nc = bacc.Bacc("TRN2", target_bir_lowering=False)
    xin = nc.declare_dram_parameter("xin", [B, F], F32, isOutput=False)
    wsh = nc.declare_dram_parameter("wsh", [CSP, F], F32, isOutput=False)
    outs = nc.declare_dram_parameter("outs", [B, CSP], F32, isOutput=True)
    sexp = nc.declare_dram_parameter("sexp", [B, 1], F32, isOutput=True)
    ident_in = nc.declare_dram_parameter("ident", [128, 128], F32,
                                         isOutput=False)

    def rsqrt_newton(pool, ss, ncols):
        """y ~= 1/sqrt(ss) for [128, ncols] ss (clamped), via
        reciprocal + sqrt + 2 Newton iterations."""
        t0 = pool.tile([128, ncols], F32, tag="nrm_t0", name="nrm_t0")
        nc.vector.tensor_scalar_max(t0, ss, 1e-24)   # ref: max(norm,1e-12)
        r0 = pool.tile([128, ncols], F32, tag="nrm_r0", name="nrm_r0")
        nc.scalar.activation(r0, t0, ACT_FN.Ln)
        y = pool.tile([128, ncols], F32, tag="nrm_y", name="nrm_y")
        # y0 = exp(-0.5*ln(ss)) ~ 1/sqrt(ss); Ln/Exp share one ACT table set
        nc.scalar.activation(y, r0, ACT_FN.Exp, scale=-0.5)
        for _ in range(2):
            a = pool.tile([128, ncols], F32, tag="nrm_a", name="nrm_a")
            nc.vector.tensor_mul(a, y, y)            # y^2
            nc.vector.tensor_mul(a, a, t0)           # ss*y^2
            nc.vector.tensor_scalar(a, a, -0.5, 1.5, ALU.mult, ALU.add)
            y2 = pool.tile([128, ncols], F32, tag="nrm_y", name="nrm_y2")
            nc.vector.tensor_mul(y2, y, a)           # y *= 1.5 - .5*ss*y^2
            y = y2
        return y

    # split mode: two operand variants (hi, lo); otherwise one
    nvar = 2 if mode == "split" else 1
    op_dt = F32 if mode == "f32" else F32R

    with tile.TileContext(nc) as tc:
        with ExitStack() as ctx:
            singles = ctx.enter_context(tc.tile_pool(name="singles", bufs=1))
            small = ctx.enter_context(tc.tile_pool(name="small", bufs=4))
            wdma = ctx.enter_context(tc.tile_pool(name="wdma", bufs=10))
            wnorm = ctx.enter_context(tc.tile_pool(name="wnorm", bufs=8))
            wtile = ctx.enter_context(tc.tile_pool(name="wtile", bufs=3))
            opool = ctx.enter_context(tc.tile_pool(name="opool", bufs=8))
            scr = ctx.enter_context(tc.tile_pool(name="scr", bufs=2))
            tpsum = ctx.enter_context(
                tc.tile_pool(name="tpsum", bufs=1, space="PSUM"))
            opsum = ctx.enter_context(
                tc.tile_pool(name="opsum", bufs=4, space="PSUM"))

            ident = singles.tile([128, 128], F32)
            if no_gpsimd:
                nc.sync.dma_start(out=ident, in_=ident_in[:, :])
            else:
                make_identity(nc, ident)

            # ------------- input prep: xnT[v][k] = (S * x / |x|).T ----
            xnT = [[singles.tile([128, B], op_dt, tag=f"xnT{v}_{k}",
                                 name=f"xnT{v}_{k}") for k in range(4)]
                   for v in range(nvar)]
            for m in range(4):
                xm = small.tile([128, F], F32, tag="xm", name="xm")
                nc.sync.dma_start(out=xm, in_=xin[m * 128:(m + 1) * 128, :])
                ssx = small.tile([128, 1], F32, tag="ssx", name="ssx")
                xsq = scr.tile([128, F], F32, tag="scr_dve", name="xsq")
                nc.vector.tensor_mul(xsq, xm, xm)
                nc.vector.tensor_reduce(ssx, xsq, axis=AX.X, op=ALU.add)
                rx = rsqrt_newton(small, ssx, 1)
                rxs = small.tile([128, 1], F32, tag="rxs", name="rxs")
                nc.vector.tensor_scalar_mul(rxs, rx, S)   # fold S=32
                xn = small.tile([128, F], F32, tag="xn", name="xn")
                nc.vector.tensor_scalar_mul(xn, xm, rxs)
                tp = tpsum.tile([128, 4 * W_CHUNK], F32, tag="tp", name="tpx")
                for k in range(4):
                    nc.tensor.transpose(
                        tp[:, k * 128:(k + 1) * 128],
                        xn[:, k * 128:(k + 1) * 128], ident)
                for k in range(4):
                    dst = xnT[0][k][:, m * 128:(m + 1) * 128]
                    src = tp[:, k * 128:(k + 1) * 128]
                    nc.scalar.copy(dst, src)
                    if nvar == 2:
                        nc.vector.tensor_sub(
                            xnT[1][k][:, m * 128:(m + 1) * 128], src, dst)

            # stats[m][:, ci] = sum_j exp(outs[m*128+p, ci*512+j])
            stats = [singles.tile([128, NCHUNK], F32, tag=f"st{m}",
                                  name=f"st{m}") for m in range(4)]

            # ---------------- main loop over class chunks -------------
            for ci in range(nchunks):
                W = min(W_CHUNK, CSP - ci * W_CHUNK)       # 512 or 256
                V = min(W, CS - ci * W_CHUNK)              # valid (212 last)
                nj = W // 128
                c0 = ci * W_CHUNK

                wts = []
                for j in range(nj):
                    wt = wdma.tile([128, F], F32, tag="wt", name="wt")
                    r0 = c0 + j * 128
                    nc.sync.dma_start(out=wt, in_=wsh[r0:r0 + 128, :])
                    wts.append(wt)

                ssw = small.tile([128, nj], F32, tag="ssw", name="ssw")
                for j in range(nj):
                    wsq = scr.tile([128, F], F32, tag="scr_dve", name="wsq")
                    nc.vector.tensor_mul(wsq, wts[j], wts[j])
                    nc.vector.tensor_reduce(ssw[:, j:j + 1], wsq,
                                            axis=AX.X, op=ALU.add)
                rw = rsqrt_newton(small, ssw, nj)

                wns = []
                for j in range(nj):
                    wn = wnorm.tile([128, F], F32, tag="wn", name="wn")
                    if no_gpsimd:
                        nc.vector.tensor_scalar_mul(wn, wts[j], rw[:, j:j + 1])
                    else:
                        nc.gpsimd.tensor_scalar_mul(wn, wts[j], rw[:, j:j + 1])
                    wns.append(wn)

                # transpose [128c,512f] tiles -> tp psum [128f, 4k x W]
                tp = tpsum.tile([128, 4 * W_CHUNK], F32, tag="tp", name="tpw")
                for j in range(nj):
                    for k in range(4):
                        nc.tensor.transpose(
                            tp[:, k * W_CHUNK + j * 128:
                               k * W_CHUNK + (j + 1) * 128],
                            wns[j][:, k * 128:(k + 1) * 128], ident)
                wnT = [wtile.tile([128, 4 * W_CHUNK], op_dt, tag=f"wnT{v}",
                                  name=f"wnT{v}") for v in range(nvar)]
                for k in range(4):
                    dst = wnT[0][:, k * W_CHUNK:k * W_CHUNK + W]
                    src = tp[:, k * W_CHUNK:k * W_CHUNK + W]
                    nc.scalar.copy(dst, src)
                    if nvar == 2:
                        nc.vector.tensor_sub(
                            wnT[1][:, k * W_CHUNK:k * W_CHUNK + W], src, dst)

                # products: hi@hi, hi@lo, lo@hi (split) or single
                if mode == "split":
                    prods = [(0, 0), (0, 1), (1, 0)]
                else:
                    prods = [(0, 0)]
                for m in range(4):
                    po = opsum.tile([128, W], F32, tag="po", name="po")
                    nmm = len(prods) * 4
                    i = 0
                    for (vx, vw) in prods:
                        for k in range(4):
                            nc.tensor.matmul(
                                po,
                                xnT[vx][k][:, m * 128:(m + 1) * 128],
                                wnT[vw][:, k * W_CHUNK:k * W_CHUNK + W],
                                start=(i == 0), stop=(i == nmm - 1))
                            i += 1
                    escr = scr.tile([128, W_CHUNK], F32, tag="scr_act",
                                    name="escr")
                    nc.scalar.activation(escr[:, :V], po[:, :V], ACT_FN.Exp,
                                         accum_out=stats[m][:, ci:ci + 1])
                    ot = opool.tile([128, W], F32, tag="ot", name="ot")
                    nc.scalar.copy(ot, po)
                    nc.sync.dma_start(
                        out=outs[m * 128:(m + 1) * 128, c0:c0 + W], in_=ot)

            # ---------------- finalize row exp-sums -------------------
            for m in range(4):
                sx = small.tile([128, 1], F32, tag="sx", name="sx")
                nc.vector.tensor_reduce(sx, stats[m][:, :nchunks],
                                        axis=AX.X, op=ALU.add)
                nc.sync.dma_start(out=sexp[m * 128:(m + 1) * 128, :], in_=sx)

    nc.compile()
    return nc


def _get_program():
    if "nc" not in _cached:
        _cached["nc"] = _build_program(MODE)
    return _cached["nc"]


def kernel(inputs: np.ndarray, labels: np.ndarray, weight: np.ndarray):
    x = np.ascontiguousarray(inputs, dtype=np.float32)
    w = np.ascontiguousarray(weight, dtype=np.float32)
    lab = np.asarray(labels).astype(np.int64)

    nc = _get_program()
    in_maps = []
    for s in range(NCORES):
        wshard = np.zeros((CSP, F), dtype=np.float32)
        wshard[:CS] = w[s * CS:(s + 1) * CS]
        in_maps.append({"xin": x, "wsh": wshard,
                        "ident": np.eye(128, dtype=np.float32)})

    res = run_bass_kernel_spmd(nc, in_maps, list(range(NCORES))).results

    outs_full = np.empty((B, C), dtype=np.float32)
    sum_exp = np.zeros(B, dtype=np.float64)
    for s in range(NCORES):
        outs_full[:, s * CS:(s + 1) * CS] = res[s]["outs"][:, :CS]
        sum_exp += res[s]["sexp"][:, 0].astype(np.float64)

    # ---- host-side label patch + loss (O(batch) work) ----
    n = np.arange(B)
    v_unp = outs_full[n, lab].astype(np.float64)          # 32*cos_in (raw)
    cos_in = np.clip(v_unp / S, -1.0 + 1e-6, 1.0 - 1e-6)
    theta = np.arccos(cos_in)
    theta_out = theta + M2
    cos_out = np.where(theta_out < math.pi,
                       cos_in * math.cos(M2) - np.sqrt(1.0 - cos_in * cos_in)
                       * math.sin(M2),
                       cos_in - MM)
    v_pat = (S * cos_out).astype(np.float32)
    outs_full[n, lab] = v_pat
    sum_exp += np.exp(v_pat.astype(np.float64)) - np.exp(v_unp)
    loss = np.float32(np.mean(np.log(sum_exp) - v_pat.astype(np.float64)))

    return (loss,
            outs_full,
            cos_in.astype(np.float32)[:, None],
            theta.astype(np.float32)[:, None],
            cos_out.astype(np.float32)[:, None])
